# revision 2
# baseline (speedup 1.0000x reference)
"""Batch semi-hard triplet loss (cosine distance) on 8 Trainium2 NeuronCores.

Design: pole-shifted reciprocal ("u-space") with single bf16 min-tree:
  - Host: sort rows by label; core c owns sorted rows [1024c, 1024(c+1)).
    Columns rotated per core so own rows sit at cols [0, 1024).  8 M-tiles
    of 128 consecutive rows per core (no padding).  Masks from labels only.
  - Device: normalize columns once (sq -> n2 -> rsqrt -> broadcast -> xtn).
    Prologue: per M-tile pole t_p = min(min dot over own-class cols, 1) via
    fused tensor_tensor_reduce over poisoned diag tiles (accum-min).
    Main loop per M-tile (pole pipelined one M-tile ahead): matmul dots
    into [128,2048] PSUM groups; ACT writes u = 1/(dot - t_p) as bf16
    (raw Reciprocal with per-partition bias); own-class dots zeroed via
    DVE mask-mult on diag slices (their u overwritten from the masked
    copy).  bf16 min-tree over u -> r1; min u = 1/(closest dot below the
    pole), or 1/(max dot - t_p) if none below (the reciprocal's
    monotonicity gives the hardest-negative fallback for free).
  - Per-row loss = relu(1/r1 + margin): d_ap - d_an + margin telescopes
    for both the semi-hard and the fallback branch.
  - Host: scatter per-row losses, mask validity by class counts, mean.
"""

import numpy as np
import ml_dtypes

B = 8192
D = 128
MARGIN = 0.2
NCORES = 8
NT = 512            # N-tile width
N_NT = B // NT      # 16
MT = 128            # M-tile rows
NMT = 8             # M-tiles per core
GW = 2048           # PSUM group width (4 banks)
NGRP = B // GW      # 4 groups per M-tile

BF16 = ml_dtypes.bfloat16

_CACHE = {}


# --------------------------------------------------------------------------
# host-side planning (pure layout, computed from labels)
# --------------------------------------------------------------------------
def _plan(labels: np.ndarray):
    order = np.argsort(labels, kind="stable")
    slab = labels[order]
    bounds = np.flatnonzero(np.r_[True, slab[1:] != slab[:-1], True])
    cls_of = np.zeros(B, np.int64)
    for i, (s, e) in enumerate(zip(bounds[:-1], bounds[1:])):
        cls_of[s:e] = i
    cls_start = bounds[:-1][cls_of]   # per sorted row: class start (global)
    cls_end = bounds[1:][cls_of]

    # per core/row: class interval(s) in rotated coords (may wrap)
    # rotated col of global sorted col g for core c: (g - 1024c) mod B
    # diag tiles per M-tile: union over cores of touched N-tiles
    diag = [set() for _ in range(NMT)]
    for c in range(NCORES):
        base = 1024 * c
        for m in range(NMT):
            rows = np.arange(base + m * MT, base + (m + 1) * MT)
            s = (cls_start[rows] - base) % B
            e_incl = (cls_end[rows] - 1 - base) % B    # inclusive end
            for si, ei in zip(s, e_incl):
                if si <= ei:
                    diag[m].add(int(si // NT))
                    diag[m].add(int(ei // NT))
                else:                  # wrapped interval
                    diag[m].add(int(si // NT))
                    diag[m].add(15)
                    diag[m].add(0)
                    diag[m].add(int(ei // NT))
    diag = [sorted(d) for d in diag]
    return dict(order=order, slab=slab, cls_start=cls_start, cls_end=cls_end,
                diag=diag, nd=sum(len(d) for d in diag))


def _build_core_inputs(emb_sorted: np.ndarray, plan, c: int):
    """xt_rot [D, B] bf16, npcat [128, ND*NT] bf16, spcat [128, ND*NT] bf16."""
    cls_start, cls_end = plan["cls_start"], plan["cls_end"]
    diag = plan["diag"]
    base = 1024 * c
    rot = (np.arange(B) + base) % B           # col j <- sorted row base+j
    xt_rot = np.ascontiguousarray(emb_sorted[rot].T).astype(BF16)

    nd = plan["nd"]
    npcat = np.full((MT, nd * NT), 1.0e4, np.float32)   # pos-min: off-class big
    spcat = np.ones((MT, nd * NT), np.float32)          # dot mask: 0 on class
    bi = 0
    for m in range(NMT):
        rows = np.arange(base + m * MT, base + (m + 1) * MT)
        s_loc = (cls_start[rows] - base) % B
        e_loc = (cls_end[rows] - 1 - base) % B          # inclusive
        for d in diag[m]:
            lo, hi = d * NT, (d + 1) * NT
            for r in range(MT):
                si, ei = int(s_loc[r]), int(e_loc[r])
                if si <= ei:
                    ivs = [(si, ei)]
                else:
                    ivs = [(si, B - 1), (0, ei)]
                for a, b_ in ivs:
                    a2, b2 = max(a, lo), min(b_, hi - 1)
                    if a2 <= b2:
                        npcat[r, bi * NT + (a2 - lo): bi * NT + (b2 - lo) + 1] = 0.0
                        spcat[r, bi * NT + (a2 - lo): bi * NT + (b2 - lo) + 1] = 0.0
            bi += 1
    return xt_rot, npcat.astype(BF16), spcat.astype(BF16)


# --------------------------------------------------------------------------
# device program
# --------------------------------------------------------------------------
def _raw_recip_bias(nc, out, in_, bias_ap):
    """out = 1 / (in_ + bias) (per-partition bias AP) on the Activation engine."""
    import concourse.mybir as mybir

    eng = nc.scalar
    ins = [
        eng.lower_ap(in_),
        eng.lower_ap(bias_ap),
        mybir.ImmediateValue(dtype=mybir.dt.float32, value=1.0),  # scale
        mybir.ImmediateValue(dtype=mybir.dt.float32, value=0.0),  # alpha
    ]
    return eng.add_instruction(
        mybir.InstActivation(
            name=f"I-{nc.next_id()}",
            func=mybir.ActivationFunctionType.Reciprocal,
            ins=ins,
            outs=[eng.lower_ap(out)],
        )
    )


def _build_bass(diag, nd, limit=None):
    """limit: None=full, 'setup', 'prologue', or int = number of main M-tiles."""
    import concourse.bacc as bacc
    import concourse.mybir as mybir
    from concourse.tile import TileContext

    f32 = mybir.dt.float32
    bf16 = mybir.dt.bfloat16
    i16 = mybir.dt.int16
    Alu = mybir.AluOpType
    Act = mybir.ActivationFunctionType
    FMAX = 3.0e38

    nc = bacc.Bacc("TRN2", target_bir_lowering=False, debug=False, num_devices=NCORES)

    xt_d = nc.dram_tensor("xt", [D, B], bf16, kind="ExternalInput").ap()
    mk_d = nc.dram_tensor("mk", [MT, nd * NT], bf16, kind="ExternalInput").ap()
    rnsc_d = nc.dram_tensor("rnsc", [1, B], bf16, kind="Internal").ap()
    sel_d = nc.dram_tensor("selc", [D, 16 * 16 + 16 * D], bf16, kind="ExternalInput").ap()
    out_d = nc.dram_tensor("out", [MT, NMT], f32, kind="ExternalOutput").ap()

    # diag bookkeeping: flat index of (m, d)
    dflat = {}
    for m in range(NMT):
        for d in diag[m]:
            dflat[(m, d)] = len(dflat)

    with TileContext(nc) as tc:
        with (
            tc.tile_pool(name="big", bufs=1) as big,
            tc.tile_pool(name="spool", bufs=2) as spool,
            tc.tile_pool(name="tr", bufs=2) as trp,
            tc.tile_pool(name="sm", bufs=4) as smp,
            tc.tile_pool(name="psA", bufs=2, space="PSUM") as psA,
        ):
            # ---------------- load ----------------
            xt = big.tile([D, B], bf16, tag="xt")
            for j in range(4):
                sl = slice(j * (B // 4), (j + 1) * (B // 4))
                nc.sync.dma_start(xt[:, sl], xt_d[:, sl])
            # selector matrices (host constants): sel for n2 accumulation,
            # sel2 (rows 0-15) for rn partition-broadcast
            selc = big.tile([D, 16 * 16 + 16 * D], bf16, tag="selc")
            nc.sync.dma_start(selc[:], sel_d)
            # PE warmup: ramp the tensor engine while DMAs land
            wusrc = big.tile([D, NT], bf16, tag="wusrc")
            nc.vector.memset(wusrc[:, :], 1.0)
            wwu = psA.tile([D, GW], f32, tag="w")
            for _ in range(14):
                nc.tensor.matmul(wwu[0:16, 0:NT], wusrc[:, 0:16], wusrc[:],
                                 skip_group_check=True)
            # mask: spc01 (0 on class cols, 1 off); npc derived on device
            mk = big.tile([MT, nd * NT], bf16, tag="mk")
            nc.sync.dma_start(mk[:], mk_d)
            npc = big.tile([MT, nd * NT], bf16, tag="npc")

            def npc_s(j):
                return npc[:, j * NT:(j + 1) * NT]

            def spc_s(j):
                return mk[:, j * NT:(j + 1) * NT]

            stage_order = ["dma", "sq", "n2", "rn", "xtn", "prologue"]
            stop_at = limit if limit in stage_order else None

            def active(st):
                return stop_at is None or stage_order.index(st) <= stage_order.index(stop_at)

            # ---------------- normalize columns ----------------
            sq = big.tile([D, B], bf16, tag="sq")
            if active("sq"):
                for j in range(4):
                    sl = slice(j * (B // 4), (j + 1) * (B // 4))
                    nc.vector.tensor_tensor(sq[:, sl], xt[:, sl], xt[:, sl], Alu.mult)

            rnb = big.tile([16, NT], bf16, tag="rnb")
            if active("n2"):
                w_n2 = psA.tile([D, GW], f32, tag="w")
                n2p = w_n2[0:16, 0:NT]
                for t in range(16):
                    nc.tensor.matmul(
                        n2p, selc[:, 16 * t: 16 * (t + 1)],
                        sq[:, t * NT: (t + 1) * NT],
                        start=(t == 0), stop=(t == 15),
                    )
                # rsqrt: sqrt + recip + 1 NR step
                sN = big.tile([16, NT], f32, tag="sN")
                if active("rn"):
                    nc.scalar.activation(sN[:], n2p, Act.Sqrt)
                    r0 = big.tile([16, NT], f32, tag="r0")
                    nc.vector.reciprocal(r0[:], sN[:])
                    a = big.tile([16, NT], f32, tag="aN")
                    nc.vector.tensor_tensor(a[:], r0[:], r0[:], Alu.mult)
                    nc.vector.tensor_tensor(a[:], a[:], n2p, Alu.mult)
                    nc.vector.tensor_scalar(a[:], a[:], -0.5, 1.5, Alu.mult, Alu.add)
                    nc.vector.tensor_tensor(a[:], a[:], r0[:], Alu.mult)
                    nc.vector.tensor_copy(rnb[:], a[:])
                else:
                    nc.vector.tensor_copy(rnb[:], n2p)

            r1s = big.tile([MT, NMT], f32, tag="r1s")

            # broadcast rn across partitions via DRAM bounce, then 2x TTs
            xtn = big.tile([D, B], bf16, tag="xtn")
            if active("xtn"):
                nc.sync.dma_start(rnsc_d.rearrange("a (p f) -> (a p) f", p=16), rnb[:])
                nc.vector.tensor_scalar(npc[:], mk[:], 1.0e4, 0.0, Alu.mult, Alu.add)
                nc.vector.memset(r1s[:], 1.0)
                rn_sb = big.tile([D, B], bf16, tag="rn_sb")
                for g in range(NGRP):
                    gsl = slice(g * GW, (g + 1) * GW)
                    nc.sync.dma_start(
                        rn_sb[:, gsl], rnsc_d[:, gsl].broadcast_to([D, GW])
                    )
                    nc.vector.tensor_tensor(xtn[:, gsl], xt[:, gsl], rn_sb[:, gsl], Alu.mult)

            # ---------------- prologue: poles ----------------
            main_mt = NMT
            if stop_at is not None and stop_at != "prologue":
                main_mt = -1
            elif limit == "prologue":
                main_mt = 0
            elif isinstance(limit, int):
                main_mt = limit

            ntpa = big.tile([MT, NMT], f32, tag="ntpa")
            posm = big.tile([MT, max(nd, 1)], f32, tag="posm")

            morder = sorted(range(NMT), key=lambda m: (max(diag[m]) >= 8, m))

            def emit_pole(m):
                lhsT = xtn[:, m * MT:(m + 1) * MT]
                for d in diag[m]:
                    j = dflat[(m, d)]
                    wp = psA.tile([D, GW], f32, tag="w")
                    nc.tensor.matmul(wp[:, :NT], lhsT, xtn[:, d * NT:(d + 1) * NT])
                    scrap = smp.tile([MT, NT], f32, tag="scrap")
                    nc.vector.tensor_tensor(scrap[:], wp[:, :NT], npc_s(j), Alu.add)
                    nc.vector.tensor_reduce(
                        posm[:, j:j + 1], scrap[:], axis=mybir.AxisListType.X, op=Alu.min
                    )
                js = [dflat[(m, d)] for d in diag[m]]
                j0, j1 = js[0], js[-1]
                assert js == list(range(j0, j1 + 1))
                if len(js) == 1:
                    nc.vector.tensor_scalar(
                        ntpa[:, m:m + 1], posm[:, j0:j0 + 1], -1.0, -1.0,
                        Alu.mult, Alu.max,
                    )
                else:
                    red = smp.tile([MT, 1], f32, tag="red")
                    nc.vector.tensor_reduce(
                        red[:], posm[:, j0:j1 + 1], axis=mybir.AxisListType.X, op=Alu.min
                    )
                    nc.vector.tensor_scalar(
                        ntpa[:, m:m + 1], red[:], -1.0, -1.0, Alu.mult, Alu.max
                    )

            def emit_produce(m):
                lhsT = xtn[:, m * MT:(m + 1) * MT]
                u = spool.tile([MT, B], bf16, tag="u")
                ntp = ntpa[:, m:m + 1]
                half = []
                for g in range(NGRP):
                    wg = psA.tile([D, GW], f32, tag="w")
                    for k in range(GW // NT):
                        t = g * (GW // NT) + k
                        nc.tensor.matmul(
                            wg[:, k * NT:(k + 1) * NT], lhsT,
                            xtn[:, t * NT:(t + 1) * NT],
                        )
                    gsl = slice(g * GW, (g + 1) * GW)
                    _raw_recip_bias(nc, u[:, gsl], wg[:], ntp)
                    for d in diag[m]:
                        if g * (GW // NT) <= d < (g + 1) * (GW // NT):
                            j = dflat[(m, d)]
                            ko = (d - g * (GW // NT)) * NT
                            dsc = smp.tile([MT, NT], bf16, tag="dsc")
                            nc.vector.tensor_tensor(
                                dsc[:], wg[:, ko:ko + NT], spc_s(j), Alu.mult
                            )
                            dss = smp.tile([MT, NT], bf16, tag="dss")
                            nc.vector.tensor_scalar(dss[:], dsc[:], ntp, None, Alu.add)
                            with nc.allow_low_precision(reason="bf16 u"):
                                nc.vector.reciprocal(
                                    u[:, d * NT:(d + 1) * NT], dss[:]
                                )
                    if g == 1:
                        t1a = trp.tile([MT, GW], bf16, tag="t1a")
                        nc.vector.tensor_tensor(
                            t1a[:], u[:, 0:GW], u[:, GW:2 * GW], Alu.min
                        )
                        half.append(t1a)
                    elif g == 3:
                        t1b = trp.tile([MT, GW], bf16, tag="t1b")
                        nc.vector.tensor_tensor(
                            t1b[:], u[:, 2 * GW:3 * GW], u[:, 3 * GW:], Alu.min
                        )
                        half.append(t1b)
                return half

            def emit_tree(m, half):
                t1a, t1b = half
                t2 = trp.tile([MT, B // 4], bf16, tag="t2")
                nc.vector.tensor_tensor(t2[:], t1a[:], t1b[:], Alu.min)
                t3 = trp.tile([MT, B // 8], bf16, tag="t3")
                nc.vector.tensor_tensor(t3[:], t2[:, :B // 8], t2[:, B // 8:], Alu.min)
                t4 = trp.tile([MT, B // 16], bf16, tag="t4")
                nc.vector.tensor_tensor(t4[:], t3[:, :B // 16], t3[:, B // 16:], Alu.min)
                nc.vector.tensor_reduce(
                    r1s[:, m:m + 1], t4[:], axis=mybir.AxisListType.X, op=Alu.min
                )

            if main_mt >= 0:
                nmain = NMT if main_mt == NMT else max(main_mt, 0)
                if main_mt == 0:
                    for m in morder:
                        emit_pole(m)
                else:
                    todo = morder[:nmain]
                    emit_pole(todo[0])
                    for i, m in enumerate(todo):
                        if i + 1 < len(todo):
                            emit_pole(todo[i + 1])
                        half = emit_produce(m)
                        emit_tree(m, half)

            # ---------------- epilogue (batched) ----------------
            # per-row loss = relu(1/r1 + margin)
            inv = big.tile([MT, NMT], f32, tag="inv")
            nc.vector.reciprocal(inv[:], r1s[:])
            out_buf = big.tile([MT, NMT], f32, tag="outb")
            nc.vector.tensor_scalar(
                out_buf[:], inv[:], MARGIN, 0.0, Alu.add, Alu.max
            )
            nc.sync.dma_start(out_d, out_buf[:])

    nc.compile()
    return nc


# --------------------------------------------------------------------------
# entry point
# --------------------------------------------------------------------------
def _selc():
    selc = np.zeros((D, 16 * 16 + 16 * D), BF16)
    for t in range(16):
        selc[:, 16 * t + t] = 1.0               # sel: e_t columns
        selc[t, 16 * 16 + D * t: 16 * 16 + D * (t + 1)] = 1.0   # sel2 rows
    return selc


def _prepare(embeddings, labels):
    emb = np.asarray(embeddings, dtype=np.float32)
    lab = np.asarray(labels).astype(np.int64)
    plan = _plan(lab)
    emb_sorted = emb[plan["order"]]
    cores = [_build_core_inputs(emb_sorted, plan, c) for c in range(NCORES)]
    return lab, plan, cores


def _host_reduce(plan, outs):
    per_row_sorted = np.zeros(B, dtype=np.float64)
    for c in range(NCORES):
        o = np.asarray(outs[c]["out"], np.float64)      # [128, 8]
        per_row_sorted[1024 * c: 1024 * (c + 1)] = o.T.reshape(-1)
    slab = plan["slab"]
    _, counts = np.unique(slab, return_counts=True)
    cnt_of = dict(zip(_.tolist(), counts.tolist()))
    cnt_row = np.array([cnt_of[int(x)] for x in slab], dtype=np.int64)
    valid = (cnt_row >= 2) & (cnt_row <= B - 1)
    num_valid = max(int(valid.sum()), 1)
    loss = per_row_sorted[valid].sum() / num_valid
    return np.array(loss, dtype=np.float32)


def kernel_run(embeddings, labels, trace=False):
    import concourse.bass_utils as bass_utils

    lab, plan, cores = _prepare(embeddings, labels)
    key = (tuple(tuple(d) for d in plan["diag"]), plan["nd"])
    if key not in _CACHE:
        _CACHE[key] = _build_bass(plan["diag"], plan["nd"])
    nc = _CACHE[key]
    selc = _selc()
    in_maps = [
        {"xt": np.ascontiguousarray(c[0]),
         "mk": np.ascontiguousarray(c[2]),
         "selc": selc}
        for c in cores
    ]
    res = bass_utils.run_bass_kernel_spmd(
        nc, in_maps, core_ids=list(range(NCORES)), trace=trace
    )
    loss = _host_reduce(plan, res.results)
    return loss, res


def kernel(embeddings, labels):
    loss, _ = kernel_run(embeddings, labels)
    return loss


# revision 3
# speedup vs baseline: 1.1426x; 1.1426x over previous
"""Batch semi-hard triplet loss (cosine distance) on 8 Trainium2 NeuronCores.

Design: pole-shifted reciprocal ("u-space") with a single bf16 min-tree:
  - Host: sort rows by label; core c owns sorted rows [1024c, 1024(c+1)).
    Columns rotated per core so own rows sit at cols [0, 1024).  8 M-tiles
    of 128 consecutive rows per core (no padding).  Masks from labels only.
  - Device: normalize columns once (sq -> n2 -> rsqrt -> broadcast -> xtn).
    Poles t_p = min(dot over own-class cols, 1) via narrow class-window
    reduces (windows precomputed on host, core-independent after rotation),
    pipelined one M-tile ahead of the producers.
    Main loop per M-tile: matmul dots into [128,2048] PSUM groups; ACT
    writes u = 1/(dot - t_p) bf16 (raw Reciprocal, per-partition bias);
    own-class dots zeroed via DVE mask-mult limited to the class window.
    bf16 min-tree over u -> r1: min u = 1/(closest dot below pole), or
    1/(max dot - t_p) if none below (hardest-negative fallback for free).
  - Per-row loss = relu(1/r1 + margin): d_ap - d_an + margin telescopes
    for both the semi-hard and the fallback branch.
  - Host: scatter per-row losses, mask validity by class counts, mean.
"""

import numpy as np
import ml_dtypes

B = 8192
D = 128
MARGIN = 0.2
NCORES = 8
NT = 512            # N-tile width
N_NT = B // NT      # 16
MT = 128            # M-tile rows
NMT = 8             # M-tiles per core
GW = 2048           # PSUM group width (4 banks)
NGRP = B // GW      # 4 groups per M-tile

BF16 = ml_dtypes.bfloat16

_CACHE = {}


# --------------------------------------------------------------------------
# host-side planning (pure layout, computed from labels)
# --------------------------------------------------------------------------
def _plan(labels: np.ndarray):
    order = np.argsort(labels, kind="stable")
    slab = labels[order]
    bounds = np.flatnonzero(np.r_[True, slab[1:] != slab[:-1], True])
    cls_of = np.zeros(B, np.int64)
    for i, (s, e) in enumerate(zip(bounds[:-1], bounds[1:])):
        cls_of[s:e] = i
    cls_start = bounds[:-1][cls_of]   # per sorted row: class start (global)
    cls_end = bounds[1:][cls_of]

    # per core/row: class interval(s) in rotated coords (may wrap)
    # rotated col of global sorted col g for core c: (g - 1024c) mod B
    # diag tiles per M-tile: union over cores of touched N-tiles
    diag = [set() for _ in range(NMT)]
    for c in range(NCORES):
        base = 1024 * c
        for m in range(NMT):
            rows = np.arange(base + m * MT, base + (m + 1) * MT)
            s = (cls_start[rows] - base) % B
            e_incl = (cls_end[rows] - 1 - base) % B    # inclusive end
            for si, ei in zip(s, e_incl):
                if si <= ei:
                    diag[m].add(int(si // NT))
                    diag[m].add(int(ei // NT))
                else:                  # wrapped interval
                    diag[m].add(int(si // NT))
                    diag[m].add(15)
                    diag[m].add(0)
                    diag[m].add(int(ei // NT))
    diag = [sorted(d) for d in diag]
    # per-(m, d): union window of class cols within tile d (in-tile coords)
    win = {}
    for c in range(NCORES):
        base = 1024 * c
        for m in range(NMT):
            rows = np.arange(base + m * MT, base + (m + 1) * MT)
            s = (cls_start[rows] - base) % B
            e_incl = (cls_end[rows] - 1 - base) % B
            for si, ei in zip(s, e_incl):
                ivs = [(si, ei)] if si <= ei else [(si, B - 1), (0, ei)]
                for a, b_ in ivs:
                    for d in diag[m]:
                        lo, hi = d * NT, (d + 1) * NT - 1
                        a2, b2 = max(int(a), lo), min(int(b_), hi)
                        if a2 <= b2:
                            w = win.get((m, d))
                            win[(m, d)] = (min(w[0], a2 - lo) if w else a2 - lo,
                                           max(w[1], b2 - lo) if w else b2 - lo)
    # pad/round windows: [lo, hi] inclusive -> slice [lo, hi+1)
    win = {k: (v[0], v[1] + 1) for k, v in win.items()}
    for m in range(NMT):
        for d in diag[m]:
            win.setdefault((m, d), (0, 1))
    return dict(order=order, slab=slab, cls_start=cls_start, cls_end=cls_end,
                diag=diag, nd=sum(len(d) for d in diag), win=win)


def _build_core_inputs(emb_sorted: np.ndarray, plan, c: int):
    """xt_rot [D, B] bf16, npcat [128, ND*NT] bf16, spcat [128, ND*NT] bf16."""
    cls_start, cls_end = plan["cls_start"], plan["cls_end"]
    diag = plan["diag"]
    base = 1024 * c
    rot = (np.arange(B) + base) % B           # col j <- sorted row base+j
    xt_rot = np.ascontiguousarray(emb_sorted[rot].T).astype(BF16)

    nd = plan["nd"]
    npcat = np.full((MT, nd * NT), 1.0e4, np.float32)   # pos-min: off-class big
    spcat = np.ones((MT, nd * NT), np.float32)          # dot mask: 0 on class
    bi = 0
    for m in range(NMT):
        rows = np.arange(base + m * MT, base + (m + 1) * MT)
        s_loc = (cls_start[rows] - base) % B
        e_loc = (cls_end[rows] - 1 - base) % B          # inclusive
        for d in diag[m]:
            lo, hi = d * NT, (d + 1) * NT
            for r in range(MT):
                si, ei = int(s_loc[r]), int(e_loc[r])
                if si <= ei:
                    ivs = [(si, ei)]
                else:
                    ivs = [(si, B - 1), (0, ei)]
                for a, b_ in ivs:
                    a2, b2 = max(a, lo), min(b_, hi - 1)
                    if a2 <= b2:
                        npcat[r, bi * NT + (a2 - lo): bi * NT + (b2 - lo) + 1] = 0.0
                        spcat[r, bi * NT + (a2 - lo): bi * NT + (b2 - lo) + 1] = 0.0
            bi += 1
    return xt_rot, npcat.astype(BF16), spcat.astype(BF16)


# --------------------------------------------------------------------------
# device program
# --------------------------------------------------------------------------
def _raw_recip_bias(nc, out, in_, bias_ap):
    """out = 1 / (in_ + bias) (per-partition bias AP) on the Activation engine."""
    import concourse.mybir as mybir

    eng = nc.scalar
    ins = [
        eng.lower_ap(in_),
        eng.lower_ap(bias_ap),
        mybir.ImmediateValue(dtype=mybir.dt.float32, value=1.0),  # scale
        mybir.ImmediateValue(dtype=mybir.dt.float32, value=0.0),  # alpha
    ]
    return eng.add_instruction(
        mybir.InstActivation(
            name=f"I-{nc.next_id()}",
            func=mybir.ActivationFunctionType.Reciprocal,
            ins=ins,
            outs=[eng.lower_ap(out)],
        )
    )


def _build_bass(diag, nd, win=None, limit=None):
    """limit: None=full, 'setup', 'prologue', or int = number of main M-tiles."""
    import concourse.bacc as bacc
    import concourse.mybir as mybir
    from concourse.tile import TileContext

    f32 = mybir.dt.float32
    bf16 = mybir.dt.bfloat16
    i16 = mybir.dt.int16
    Alu = mybir.AluOpType
    Act = mybir.ActivationFunctionType
    FMAX = 3.0e38

    nc = bacc.Bacc("TRN2", target_bir_lowering=False, debug=False, num_devices=NCORES)

    xt_d = nc.dram_tensor("xt", [D, B], bf16, kind="ExternalInput").ap()
    mk_d = nc.dram_tensor("mk", [MT, nd * NT], bf16, kind="ExternalInput").ap()
    rnsc_d = nc.dram_tensor("rnsc", [1, B], bf16, kind="Internal").ap()
    sel_d = nc.dram_tensor("selc", [D, 16 * 16 + 16 * D], bf16, kind="ExternalInput").ap()
    out_d = nc.dram_tensor("out", [MT, NMT], f32, kind="ExternalOutput").ap()

    if win is None:
        win = {(m, d): (0, NT) for m in range(NMT) for d in diag[m]}
    # diag bookkeeping: flat index of (m, d)
    dflat = {}
    for m in range(NMT):
        for d in diag[m]:
            dflat[(m, d)] = len(dflat)

    with TileContext(nc) as tc:
        with (
            tc.tile_pool(name="big", bufs=1) as big,
            tc.tile_pool(name="spool", bufs=2) as spool,
            tc.tile_pool(name="tr", bufs=2) as trp,
            tc.tile_pool(name="sm", bufs=4) as smp,
            tc.tile_pool(name="psA", bufs=2, space="PSUM") as psA,
        ):
            # ---------------- load ----------------
            xt = big.tile([D, B], bf16, tag="xt")
            for j in range(4):
                sl = slice(j * (B // 4), (j + 1) * (B // 4))
                nc.sync.dma_start(xt[:, sl], xt_d[:, sl])
            # selector matrices (host constants): sel for n2 accumulation,
            # sel2 (rows 0-15) for rn partition-broadcast
            selc = big.tile([D, 16 * 16 + 16 * D], bf16, tag="selc")
            nc.sync.dma_start(selc[:], sel_d)
            # PE warmup: ramp the tensor engine while DMAs land
            wusrc = big.tile([D, NT], bf16, tag="wusrc")
            nc.vector.memset(wusrc[:, :], 1.0)
            wwu = psA.tile([D, GW], f32, tag="w")
            for _ in range(14):
                nc.tensor.matmul(wwu[0:16, 0:NT], wusrc[:, 0:16], wusrc[:],
                                 skip_group_check=True)
            # mask: spc01 (0 on class cols, 1 off); npc derived on device
            mk = big.tile([MT, nd * NT], bf16, tag="mk")
            nc.sync.dma_start(mk[:], mk_d)
            npc = big.tile([MT, nd * NT], bf16, tag="npc")

            def npc_s(j):
                return npc[:, j * NT:(j + 1) * NT]

            def spc_s(j):
                return mk[:, j * NT:(j + 1) * NT]

            stage_order = ["dma", "sq", "n2", "rn", "xtn", "prologue"]
            stop_at = limit if limit in stage_order else None

            def active(st):
                return stop_at is None or stage_order.index(st) <= stage_order.index(stop_at)

            # ---------------- normalize columns ----------------
            sq = big.tile([D, B], bf16, tag="sq")
            if active("sq"):
                for j in range(4):
                    sl = slice(j * (B // 4), (j + 1) * (B // 4))
                    nc.vector.tensor_tensor(sq[:, sl], xt[:, sl], xt[:, sl], Alu.mult)

            rnb = big.tile([16, NT], bf16, tag="rnb")
            if active("n2"):
                w_n2 = psA.tile([D, GW], f32, tag="w")
                n2p = w_n2[0:16, 0:NT]
                for t in range(16):
                    nc.tensor.matmul(
                        n2p, selc[:, 16 * t: 16 * (t + 1)],
                        sq[:, t * NT: (t + 1) * NT],
                        start=(t == 0), stop=(t == 15),
                    )
                # rsqrt: rn = sqrt(1/n2)  (DVE reciprocal is the accurate one)
                if active("rn"):
                    r0 = big.tile([16, NT], f32, tag="r0")
                    nc.vector.reciprocal(r0[:], n2p)
                    nc.scalar.activation(rnb[:], r0[:], Act.Sqrt)
                    # preload the Reciprocal act table during setup idle
                    dum = smp.tile([MT, 8], f32, tag="dum")
                    _raw_recip_bias(nc, dum[:], wusrc[:, 0:8], wusrc[:, 8:9])
                else:
                    nc.vector.tensor_copy(rnb[:], n2p)

            r1s = big.tile([MT, NMT], f32, tag="r1s")

            # broadcast rn across partitions via DRAM bounce, then 2x TTs
            xtn = big.tile([D, B], bf16, tag="xtn")
            if active("xtn"):
                nc.sync.dma_start(rnsc_d.rearrange("a (p f) -> (a p) f", p=16), rnb[:])
                nc.vector.tensor_scalar(npc[:], mk[:], 1.0e4, 0.0, Alu.mult, Alu.add)
                nc.vector.memset(r1s[:], 1.0)
                rn_sb = big.tile([D, B], bf16, tag="rn_sb")
                xtn_done = [False] * NGRP

                def emit_xtn(g):
                    if xtn_done[g]:
                        return
                    xtn_done[g] = True
                    gsl = slice(g * GW, (g + 1) * GW)
                    nc.sync.dma_start(
                        rn_sb[:, gsl], rnsc_d[:, gsl].broadcast_to([D, GW])
                    )
                    nc.vector.tensor_tensor(xtn[:, gsl], xt[:, gsl], rn_sb[:, gsl], Alu.mult)
            else:
                def emit_xtn(g):
                    pass

            # ---------------- prologue: poles ----------------
            main_mt = NMT
            if stop_at is not None and stop_at != "prologue":
                main_mt = -1
            elif limit == "prologue":
                main_mt = 0
            elif isinstance(limit, int):
                main_mt = limit

            ntpa = big.tile([MT, NMT], f32, tag="ntpa")
            posm = big.tile([MT, max(nd, 1)], f32, tag="posm")

            morder = sorted(range(NMT), key=lambda m: (max(diag[m]) >= 8, m))

            def emit_pole(m):
                lhsT = xtn[:, m * MT:(m + 1) * MT]
                for d in diag[m]:
                    j = dflat[(m, d)]
                    lo, hi = win[(m, d)]
                    wd = hi - lo
                    wp = psA.tile([D, GW], f32, tag="w")
                    nc.tensor.matmul(
                        wp[:, :wd], lhsT, xtn[:, d * NT + lo:d * NT + hi]
                    )
                    scrap = smp.tile([MT, NT], f32, tag="scrap")
                    nc.vector.tensor_tensor(
                        scrap[:, :wd], wp[:, :wd],
                        npc[:, j * NT + lo:j * NT + hi], Alu.add
                    )
                    nc.vector.tensor_reduce(
                        posm[:, j:j + 1], scrap[:, :wd],
                        axis=mybir.AxisListType.X, op=Alu.min
                    )
                js = [dflat[(m, d)] for d in diag[m]]
                j0, j1 = js[0], js[-1]
                assert js == list(range(j0, j1 + 1))
                if len(js) == 1:
                    nc.vector.tensor_scalar(
                        ntpa[:, m:m + 1], posm[:, j0:j0 + 1], -1.0, -1.0,
                        Alu.mult, Alu.max,
                    )
                else:
                    red = smp.tile([MT, 1], f32, tag="red")
                    nc.vector.tensor_reduce(
                        red[:], posm[:, j0:j1 + 1], axis=mybir.AxisListType.X, op=Alu.min
                    )
                    nc.vector.tensor_scalar(
                        ntpa[:, m:m + 1], red[:], -1.0, -1.0, Alu.mult, Alu.max
                    )

            def emit_produce(m):
                lhsT = xtn[:, m * MT:(m + 1) * MT]
                u = spool.tile([MT, B], bf16, tag="u")
                ntp = ntpa[:, m:m + 1]
                half = []
                for g in range(NGRP):
                    wg = psA.tile([D, GW], f32, tag="w")
                    for k in range(GW // NT):
                        t = g * (GW // NT) + k
                        nc.tensor.matmul(
                            wg[:, k * NT:(k + 1) * NT], lhsT,
                            xtn[:, t * NT:(t + 1) * NT],
                        )
                    gsl = slice(g * GW, (g + 1) * GW)
                    _raw_recip_bias(nc, u[:, gsl], wg[:], ntp)
                    for d in diag[m]:
                        if g * (GW // NT) <= d < (g + 1) * (GW // NT):
                            j = dflat[(m, d)]
                            lo, hi = win[(m, d)]
                            wd = hi - lo
                            ko = (d - g * (GW // NT)) * NT + lo
                            dsc = smp.tile([MT, NT], bf16, tag="dsc")
                            nc.vector.tensor_tensor(
                                dsc[:, :wd], wg[:, ko:ko + wd],
                                mk[:, j * NT + lo:j * NT + hi], Alu.mult
                            )
                            dss = smp.tile([MT, NT], bf16, tag="dss")
                            nc.vector.tensor_scalar(
                                dss[:, :wd], dsc[:, :wd], ntp, None, Alu.add
                            )
                            with nc.allow_low_precision(reason="bf16 u"):
                                nc.vector.reciprocal(
                                    u[:, d * NT + lo:d * NT + hi], dss[:, :wd]
                                )
                    if g == 1:
                        t1a = trp.tile([MT, GW], bf16, tag="t1a")
                        nc.vector.tensor_tensor(
                            t1a[:], u[:, 0:GW], u[:, GW:2 * GW], Alu.min
                        )
                        half.append(t1a)
                    elif g == 3:
                        t1b = trp.tile([MT, GW], bf16, tag="t1b")
                        nc.vector.tensor_tensor(
                            t1b[:], u[:, 2 * GW:3 * GW], u[:, 3 * GW:], Alu.min
                        )
                        half.append(t1b)
                return half

            def emit_tree(m, half):
                t1a, t1b = half
                t2 = trp.tile([MT, B // 4], bf16, tag="t2")
                nc.vector.tensor_tensor(t2[:], t1a[:], t1b[:], Alu.min)
                t3 = trp.tile([MT, B // 8], bf16, tag="t3")
                nc.vector.tensor_tensor(t3[:], t2[:, :B // 8], t2[:, B // 8:], Alu.min)
                t4 = trp.tile([MT, B // 16], bf16, tag="t4")
                nc.vector.tensor_tensor(t4[:], t3[:, :B // 16], t3[:, B // 16:], Alu.min)
                nc.vector.tensor_reduce(
                    r1s[:, m:m + 1], t4[:], axis=mybir.AxisListType.X, op=Alu.min
                )

            if main_mt >= 0:
                nmain = NMT if main_mt == NMT else max(main_mt, 0)
                if main_mt == 0:
                    for m in morder:
                        emit_pole(m)
                else:
                    todo = morder[:nmain]
                    for g in range(NGRP):
                        emit_xtn(g)
                    emit_pole(todo[0])
                    for i, m in enumerate(todo):
                        if i + 1 < len(todo):
                            emit_pole(todo[i + 1])
                        half = emit_produce(m)
                        emit_tree(m, half)

            # ---------------- epilogue (batched) ----------------
            # per-row loss = relu(1/r1 + margin)
            inv = big.tile([MT, NMT], f32, tag="inv")
            nc.vector.reciprocal(inv[:], r1s[:])
            out_buf = big.tile([MT, NMT], f32, tag="outb")
            nc.vector.tensor_scalar(
                out_buf[:], inv[:], MARGIN, 0.0, Alu.add, Alu.max
            )
            nc.sync.dma_start(out_d, out_buf[:])

    nc.compile()
    return nc


# --------------------------------------------------------------------------
# entry point
# --------------------------------------------------------------------------
def _selc():
    selc = np.zeros((D, 16 * 16 + 16 * D), BF16)
    for t in range(16):
        selc[:, 16 * t + t] = 1.0               # sel: e_t columns
        selc[t, 16 * 16 + D * t: 16 * 16 + D * (t + 1)] = 1.0   # sel2 rows
    return selc


def _prepare(embeddings, labels):
    emb = np.asarray(embeddings, dtype=np.float32)
    lab = np.asarray(labels).astype(np.int64)
    plan = _plan(lab)
    emb_sorted = emb[plan["order"]]
    cores = [_build_core_inputs(emb_sorted, plan, c) for c in range(NCORES)]
    return lab, plan, cores


def _host_reduce(plan, outs):
    per_row_sorted = np.zeros(B, dtype=np.float64)
    for c in range(NCORES):
        o = np.asarray(outs[c]["out"], np.float64)      # [128, 8]
        per_row_sorted[1024 * c: 1024 * (c + 1)] = o.T.reshape(-1)
    slab = plan["slab"]
    _, counts = np.unique(slab, return_counts=True)
    cnt_of = dict(zip(_.tolist(), counts.tolist()))
    cnt_row = np.array([cnt_of[int(x)] for x in slab], dtype=np.int64)
    valid = (cnt_row >= 2) & (cnt_row <= B - 1)
    num_valid = max(int(valid.sum()), 1)
    loss = per_row_sorted[valid].sum() / num_valid
    return np.array(loss, dtype=np.float32)


def kernel_run(embeddings, labels, trace=False):
    import concourse.bass_utils as bass_utils

    lab, plan, cores = _prepare(embeddings, labels)
    key = (tuple(tuple(d) for d in plan["diag"]), plan["nd"],
           tuple(sorted(plan["win"].items())))
    if key not in _CACHE:
        _CACHE[key] = _build_bass(plan["diag"], plan["nd"], win=plan["win"])
    nc = _CACHE[key]
    selc = _selc()
    in_maps = [
        {"xt": np.ascontiguousarray(c[0]),
         "mk": np.ascontiguousarray(c[2]),
         "selc": selc}
        for c in cores
    ]
    res = bass_utils.run_bass_kernel_spmd(
        nc, in_maps, core_ids=list(range(NCORES)), trace=trace
    )
    loss = _host_reduce(plan, res.results)
    return loss, res


def kernel(embeddings, labels):
    loss, _ = kernel_run(embeddings, labels)
    return loss
